# revision 12
# baseline (speedup 1.0000x reference)
"""Trainium2 Bass kernel for the spiking autoencoder (histogram_binning).

Strategy (pure data parallel across 8 NeuronCores, no collectives):
  - Each core gets a 2048-row shard of `features`; tiny weights replicated.
  - Input layer spikes have a closed form: with m = floor((x-bin0)/h) the
    spike at step k is  s_in[k] = [ (k*m mod 16) < m ]  (Bresenham).
  - All layouts are hidden-major (h on partitions); matmuls in f16 with f32
    PSUM accumulation. Membranes LIVE in PSUM: the PE accumulates currents,
    an ACT sigmoid with huge scale computes the exact {0,1} threshold, and a
    -I identity-matmul subtracts the spike (membrane reset) on the PE.
  - Layer 3 uses s2 as the stationary operand so output is sample-major.
    The output count is recovered from the final membrane identity
      sum_k s3 = b3 + (sum_k s2) @ W3eff.T - t3_final
    and rounded to the nearest integer (counts are integers), so the output
    is exactly 0 where the reference is exactly 0.
"""

import os
import numpy as np

N_CORES = 8
B, IN_DIM, HID = 16384, 784, 128
BITS = 16
NSH = B // N_CORES          # 2048 rows per core
NT = 512                    # samples per n-tile
N_TILES = NSH // NT         # 4
NSUB = NT // 128            # 4 sample-subtiles per n-tile
IN_CH = 7                   # feature chunks
CH = 128                    # chunk width (feature dim padded to 896)
IN_P = IN_CH * CH           # 896 padded feature dim
SIG_SCALE = 8192.0          # sigmoid step sharpness

_CACHE = {}




def _build(bin0, inv_h, out_scale):
    import concourse.bass as bass
    import concourse.bacc as bacc
    import concourse.mybir as mybir
    from concourse.tile import TileContext
    from contextlib import ExitStack

    f32 = mybir.dt.float32
    f16 = mybir.dt.float16
    AF = mybir.ActivationFunctionType
    OP = mybir.AluOpType

    nc = bacc.Bacc()
    feats = nc.dram_tensor("features", [NSH, IN_DIM], f32, kind="ExternalInput")
    w0t = nc.dram_tensor("w0t", [IN_P, HID], f16, kind="ExternalInput")
    w1t = nc.dram_tensor("w1t", [HID, HID], f16, kind="ExternalInput")
    w2t = nc.dram_tensor("w2t", [HID, HID], f16, kind="ExternalInput")
    w3r = nc.dram_tensor("w3r", [HID, IN_DIM], f16, kind="ExternalInput")
    b0r = nc.dram_tensor("b0r", [1, HID], f32, kind="ExternalInput")
    b1r = nc.dram_tensor("b1r", [1, HID], f32, kind="ExternalInput")
    b2r = nc.dram_tensor("b2r", [1, HID], f32, kind="ExternalInput")
    b3r = nc.dram_tensor("b3r", [1, IN_DIM], f32, kind="ExternalInput")
    constf = nc.dram_tensor("constf", [1, IN_DIM], f32, kind="ExternalInput")
    outd = nc.dram_tensor("out", [NSH, IN_DIM], f32, kind="ExternalOutput")

    ctx = ExitStack()
    with ctx:
        tc = ctx.enter_context(TileContext(nc))
        consts = ctx.enter_context(tc.tile_pool(name="consts", bufs=1))
        featp = ctx.enter_context(tc.tile_pool(name="featp", bufs=3))
        mgen = ctx.enter_context(tc.tile_pool(name="mgen", bufs=2))
        mtp = ctx.enter_context(tc.tile_pool(name="mtp", bufs=2))
        sinp = ctx.enter_context(tc.tile_pool(name="sinp", bufs=3))
        sp = ctx.enter_context(tc.tile_pool(name="sp", bufs=3))
        s2p = ctx.enter_context(tc.tile_pool(name="s2p", bufs=2))
        s3p = ctx.enter_context(tc.tile_pool(name="s3p", bufs=3))
        t3sbp = ctx.enter_context(tc.tile_pool(name="t3sbp", bufs=2))
        outp = ctx.enter_context(tc.tile_pool(name="outp", bufs=2))
        membp = ctx.enter_context(tc.tile_pool(name="membp", bufs=1, space="PSUM"))
        t3p = ctx.enter_context(tc.tile_pool(name="t3p", bufs=2, space="PSUM"))

        sb_w0t = consts.tile([CH, IN_CH, HID], f16, tag="w0t")
        nc.sync.dma_start(out=sb_w0t, in_=w0t.rearrange("(c p) m -> p c m", p=CH))
        # (chunk rows 784..895 are zero weights -> padding contributes nothing)
        sb_w1t = consts.tile([HID, HID], f16, tag="w1t")
        nc.sync.dma_start(out=sb_w1t, in_=w1t[:, :])
        sb_w2t = consts.tile([HID, HID], f16, tag="w2t")
        nc.sync.dma_start(out=sb_w2t, in_=w2t[:, :])
        sb_w3r = consts.tile([HID, IN_DIM], f16, tag="w3r")
        nc.sync.dma_start(out=sb_w3r, in_=w3r[:, :])
        sb_b0 = consts.tile([1, HID], f32, tag="b0")
        nc.sync.dma_start(out=sb_b0, in_=b0r[:, :])
        sb_b1 = consts.tile([1, HID], f32, tag="b1")
        nc.sync.dma_start(out=sb_b1, in_=b1r[:, :])
        sb_b2 = consts.tile([1, HID], f32, tag="b2")
        nc.sync.dma_start(out=sb_b2, in_=b2r[:, :])
        sb_b3 = consts.tile([1, IN_DIM], f32, tag="b3")
        nc.sync.dma_start(out=sb_b3, in_=b3r[:, :])
        sb_cf = consts.tile([1, IN_DIM], f32, tag="cf")
        nc.sync.dma_start(out=sb_cf, in_=constf[:, :])
        sb_ones = consts.tile([1, NT], f32, tag="ones")
        nc.vector.memset(sb_ones, 1.0)
        sb_ones128 = consts.tile([1, 128], f32, tag="ones128")
        nc.vector.memset(sb_ones128, 1.0)
        io_i = consts.tile([128, 128], mybir.dt.int32, tag="ioi")
        io_j = consts.tile([128, 128], mybir.dt.int32, tag="ioj")
        nc.gpsimd.iota(io_i, pattern=[[0, 128]], base=0, channel_multiplier=1)
        nc.gpsimd.iota(io_j, pattern=[[1, 128]], base=0, channel_multiplier=0)
        sb_id = consts.tile([128, 128], f16, tag="idm")
        nc.vector.tensor_tensor(out=sb_id, in0=io_i, in1=io_j, op=OP.is_equal)
        sb_nid = consts.tile([128, 128], f16, tag="nidm")
        nc.vector.tensor_scalar(out=sb_nid, in0=sb_id, scalar1=-1.0,
                                scalar2=None, op0=OP.mult)
        sb_sigb = consts.tile([128, 1], f32, tag="sigb")
        nc.vector.memset(sb_sigb, -SIG_SCALE)

        for it in range(N_TILES):
            n0 = it * NT
            # ---- phase A ----
            sb_mt = mtp.tile([CH, IN_CH, NT], f16, tag="mt")
            for sub in range(NSUB):
                ft = featp.tile([128, IN_DIM], f32, tag="feat")
                nc.sync.dma_start(
                    out=ft, in_=feats[n0 + sub * 128: n0 + (sub + 1) * 128, :])
                yt = mgen.tile([128, IN_DIM], f32, tag="y")
                nc.vector.tensor_scalar(out=yt, in0=ft, scalar1=bin0,
                                        scalar2=inv_h, op0=OP.subtract,
                                        op1=OP.mult)
                y2 = mgen.tile([128, IN_DIM], f32, tag="y2")
                nc.vector.tensor_scalar(out=y2, in0=yt, scalar1=0.0,
                                        scalar2=None, op0=OP.max)
                # f16 round trick: round(y - 0.5 + 1536) == 1536 + floor(y)
                mq = mgen.tile([128, IN_DIM], f16, tag="mq")
                nc.vector.tensor_scalar(out=mq, in0=y2, scalar1=1535.5,
                                        scalar2=None, op0=OP.add)
                mt = mgen.tile([128, IN_P], f16, tag="m")
                nc.vector.tensor_scalar(out=mt[:, :IN_DIM], in0=mq,
                                        scalar1=1536.0, scalar2=16.0,
                                        op0=OP.subtract, op1=OP.min)
                nc.vector.memset(mt[:, IN_DIM:], 0.0)
                for c in range(IN_CH):
                    nc.sync.dma_start_transpose(
                        out=sb_mt[:, c, sub * 128:(sub + 1) * 128],
                        in_=mt[:, c * CH:(c + 1) * CH])

            # ---- phase B ----
            t0 = membp.tile([128, NT], f32, tag="t0")
            t1 = membp.tile([128, NT], f32, tag="t1")
            t2 = membp.tile([128, NT], f32, tag="t2")
            nc.tensor.matmul(t0, sb_b0, sb_ones, start=True, stop=False)
            nc.tensor.matmul(t1, sb_b1, sb_ones, start=True, stop=False)
            nc.tensor.matmul(t2, sb_b2, sb_ones, start=True, stop=False)
            s2_all = s2p.tile([HID, BITS, NT], f16, tag="s2all")
            s2sum = sp.tile([HID, NT], f16, tag="s2sum")

            a_prev = None
            for k in range(1, BITS + 1):
                # a_k = 1536 + floor(k*m/16) via f16 rounding
                ak = sinp.tile([CH, IN_CH, NT], f16, tag="a%d" % (k % 2))
                nc.vector.tensor_scalar(out=ak, in0=sb_mt,
                                        scalar1=float(k) / 16.0,
                                        scalar2=1536.0 - 15.0 / 32.0,
                                        op0=OP.mult, op1=OP.add)
                st = sinp.tile([CH, IN_CH, NT], f16, tag="sin")
                if k == 1:
                    nc.vector.tensor_scalar(out=st, in0=ak, scalar1=1536.0,
                                            scalar2=None, op0=OP.subtract)
                else:
                    nc.vector.tensor_tensor(out=st, in0=ak, in1=a_prev,
                                            op=OP.subtract)
                a_prev = ak
                for c in range(IN_CH):
                    nc.tensor.matmul(t0, sb_w0t[:, c, :], st[:, c, :],
                                     start=False, stop=False)
                s0 = sp.tile([HID, NT], f16, tag="s0")
                nc.scalar.activation(out=s0, in_=t0, func=AF.Sigmoid,
                                     bias=sb_sigb[:, :], scale=SIG_SCALE)
                nc.tensor.matmul(t0, sb_nid, s0, start=False,
                                 stop=(k == BITS))
                nc.tensor.matmul(t1, sb_w1t, s0, start=False, stop=False)
                s1 = sp.tile([HID, NT], f16, tag="s1")
                nc.scalar.activation(out=s1, in_=t1, func=AF.Sigmoid,
                                     bias=sb_sigb[:, :], scale=SIG_SCALE)
                nc.tensor.matmul(t1, sb_nid, s1, start=False,
                                 stop=(k == BITS))
                nc.tensor.matmul(t2, sb_w2t, s1, start=False, stop=False)
                s2k = s2_all[:, k - 1, :]
                nc.scalar.activation(out=s2k, in_=t2, func=AF.Sigmoid,
                                     bias=sb_sigb[:, :], scale=SIG_SCALE)
                nc.tensor.matmul(t2, sb_nid, s2k, start=False,
                                 stop=(k == BITS))
                if k == 1:
                    nc.vector.tensor_copy(s2sum, s2k)
                else:
                    nc.vector.tensor_tensor(out=s2sum, in0=s2sum, in1=s2k,
                                            op=OP.add)

            # ---- phase C ----
            NSPL = [(0, 512), (512, IN_DIM - 512)]
            for sub in range(NSUB):
                t3 = t3p.tile([128, IN_DIM], f32, tag="t3")
                for o, w in NSPL:
                    nc.tensor.matmul(t3[:, o:o + w], sb_ones128,
                                     sb_b3[:, o:o + w], start=True, stop=False)
                for k in range(BITS):
                    lhs = s2_all[:, k, sub * 128:(sub + 1) * 128]
                    for o, w in NSPL:
                        nc.tensor.matmul(t3[:, o:o + w], lhs,
                                         sb_w3r[:, o:o + w], start=False,
                                         stop=False)
                    s3 = s3p.tile([128, IN_DIM], f16, tag="s3")
                    nc.scalar.activation(out=s3, in_=t3, func=AF.Sigmoid,
                                         bias=sb_sigb[:, :], scale=SIG_SCALE)
                    for o, w in NSPL:
                        nc.tensor.matmul(t3[:, o:o + w], sb_nid,
                                         s3[:, o:o + w], start=False,
                                         stop=(k == BITS - 1))
                # final membrane -> SBUF
                t3sb = t3sbp.tile([128, IN_DIM], f32, tag="t3sb")
                nc.scalar.activation(out=t3sb, in_=t3, func=AF.Copy)
                # acc = constf + s2sum_sub @ w3r   (reuses the same psum tile)
                lhs_sum = s2sum[:, sub * 128:(sub + 1) * 128]
                for o, w in NSPL:
                    nc.tensor.matmul(t3[:, o:o + w], sb_ones128,
                                     sb_cf[:, o:o + w], start=True, stop=False)
                for o, w in NSPL:
                    nc.tensor.matmul(t3[:, o:o + w], lhs_sum,
                                     sb_w3r[:, o:o + w], start=False,
                                     stop=True)
                # d = acc - t3_final (+0.5 already inside constf); floor; scale
                dt_ = outp.tile([128, IN_DIM], f32, tag="d")
                nc.vector.tensor_tensor(out=dt_, in0=t3, in1=t3sb,
                                        op=OP.subtract)
                C23 = 12582912.0  # 1.5 * 2**23: f32 add rounds to integer
                gt2 = outp.tile([128, IN_DIM], f32, tag="g2")
                nc.vector.tensor_scalar(out=gt2, in0=dt_, scalar1=C23,
                                        scalar2=None, op0=OP.add)
                ot = outp.tile([128, IN_DIM], f32, tag="of")
                nc.vector.tensor_scalar(out=ot, in0=gt2, scalar1=C23,
                                        scalar2=out_scale, op0=OP.subtract,
                                        op1=OP.mult)
                nc.sync.dma_start(
                    out=outd[n0 + sub * 128: n0 + (sub + 1) * 128, :], in_=ot)
    nc.compile()
    return nc


def _prep(inputs):
    """Host-side prep of tiny params. Returns (nc_key_scalars, per-core maps)."""
    ib0 = np.asarray(inputs["in_bins0"], np.float32)
    h_in = [float(np.asarray(inputs["in_bins%d" % i])[1]
                  - np.asarray(inputs["in_bins%d" % i])[0]) for i in range(4)]
    h_out = [float(np.asarray(inputs["out_bins%d" % i])[1]
                   - np.asarray(inputs["out_bins%d" % i])[0]) for i in range(4)]
    ratio = [h_in[i] / h_out[i] for i in range(4)]
    Weff = [np.asarray(inputs["W%d" % i], np.float32) * np.float32(ratio[i])
            for i in range(4)]
    beff = [np.asarray(inputs["b%d" % i], np.float32) * np.float32(ratio[i])
            for i in range(4)]
    common = {
        "w0t": np.ascontiguousarray(
            np.concatenate([Weff[0].T, np.zeros((112, HID), np.float32)],
                           axis=0).astype(np.float16)),
        "w1t": np.ascontiguousarray(Weff[1].T.astype(np.float16)),
        "w2t": np.ascontiguousarray(Weff[2].T.astype(np.float16)),
        "w3r": np.ascontiguousarray(Weff[3].T.astype(np.float16)),
        "b0r": beff[0].reshape(1, -1),
        "b1r": beff[1].reshape(1, -1),
        "b2r": beff[2].reshape(1, -1),
        "b3r": beff[3].reshape(1, -1),
        "constf": (beff[3] + np.float32(0.5 - 2.0 ** -11)).reshape(1, -1),
    }
    scalars = (float(ib0[0]), float(1.0 / h_in[0]), float(h_out[3]))
    return scalars, common


def _ensure_trace_hooks():
    """Register the NTFF profile hook that this image's antenv lacks."""
    import sys, types
    try:
        import antenv.axon_hooks  # noqa: F401
        return
    except ImportError:
        pass
    mod = types.ModuleType('antenv.axon_hooks')
    mod._hook = None
    def set_axon_ntff_profile_hook(h):
        mod._hook = h
    def get_axon_ntff_profile_hook():
        return mod._hook
    mod.set_axon_ntff_profile_hook = set_axon_ntff_profile_hook
    mod.get_axon_ntff_profile_hook = get_axon_ntff_profile_hook
    sys.modules['antenv.axon_hooks'] = mod
    import antenv
    antenv.axon_hooks = mod
    try:
        from trn_agent_boot.trn_boot import _ntff_profile_via_ctypes
        h = _ntff_profile_via_ctypes('/opt/axon/libaxon_pjrt.so')
        if h:
            set_axon_ntff_profile_hook(h)
    except Exception as e:
        print("trace hook setup failed:", e)
    import concourse.bass_utils as bu
    bu.upload_artifacts = lambda tmpdir: "local://" + str(tmpdir)


def kernel(**inputs):
    from concourse.bass_utils import run_bass_kernel_spmd
    if os.environ.get("KBENCH_TRACE"):
        _ensure_trace_hooks()

    scalars, common = _prep(inputs)
    if scalars not in _CACHE:
        _CACHE[scalars] = _build(*scalars)
    nc = _CACHE[scalars]

    feats = np.ascontiguousarray(np.asarray(inputs["features"], np.float32))
    in_maps = []
    for c in range(N_CORES):
        m = dict(common)
        m["features"] = feats[c * NSH:(c + 1) * NSH]
        in_maps.append(m)
    tdir = None
    if os.environ.get("KBENCH_TRACE"):
        import tempfile
        tdir = tempfile.mkdtemp(prefix="kbench_trace_")
        print("trace dir:", tdir)
    res = run_bass_kernel_spmd(nc, in_maps, core_ids=list(range(N_CORES)),
                               trace=bool(os.environ.get("KBENCH_TRACE")),
                               tmpdir=tdir)
    outs = [r["out"] for r in res.results]
    full = np.concatenate(outs, axis=0).astype(np.float32)
    if os.environ.get("KBENCH_TRACE"):
        kernel.last_exec_time_ns = res.exec_time_ns
    return full
